# revision 15
# baseline (speedup 1.0000x reference)
"""TextCNN-style conv layer (kernel sizes 3/4/5, EMB=300 -> DEPTH=256, bias,
ReLU, max-pool over time) as a Bass/Tile kernel for 8 Trainium2 NeuronCores.

Strategy: data-parallel over batch (8 samples per core), weights replicated.

Conv as dense-K matmuls over the im2col matrix Xrep[k, i] = x[i + k//300,
k%300] in fp8 e4m3 with DoubleRow matmuls: each MM contracts a PAIR of
128-row K-subtiles (virtual K=256, two fp8 weights per PE cell), so each
branch needs 4/5/6 pair-MMs = 15 per (sample, depth-half), 240 per core --
half the bf16 count, and HW paces them at the same ~165ns (N/2.4GHz), so
the PE stream floor drops from 79us to 39.5us.  e4m3 on both operands
measures L2 rel err 1.25e-2 on the seed-0 data (gate 2e-2).

Schedule (v5, from traces of the bf16/v1/v2/v3/v4 runs):
- Per-ring DMA completions pipeline at ~size/175GB/s + ~0.7us issue after
  a ~2.5us first-fill, so the prefetch uses few, deadline-ordered chunks:
  sample-0 x in 3 on sync, weights in 3+2 on scalar, sample-1 split
  across both rings, the rest alternating whole-sample loads.  (SWDGE
  for weights regressed 4us; bias only there.)
- 5 throwaway bf16 matmuls (N=512) cover the DMA lead-in from the first
  post-barrier slot (~7.5us) to the first real MM (~10.4us); small N=256
  fillers sit at the known supply seams inside sample 0/1 so the PE
  never idles long enough to hold the HAM clock-gate at K=4/8 (a ~1us
  gap there previously delayed K=8/8 by ~2.5us of half-clock matmuls).
- PSUM: 7 banks round-robin the 48 accumulation groups, 1 for dummies.
- Per-sample epilogue: bias-add + relu + own small contiguous output
  DMA, so the post-stream tail is one reduce + tiny DVE ops + one DMA.
- Fixed framework cost ~9us (semaphore-sweep teardown + entry barriers)
  is the same for any kernel here (57-59 sems/engine swept).
"""

import numpy as np
import ml_dtypes

B, SEQ, EMB = 64, 394, 300
DEPTH = 256
NCORES = 8
BPC = B // NCORES  # samples per core
SEQP = 400  # im2col free-dim padded (zeros) so all windows exist
NS = (3, 4, 5)
NPAIRS = (4, 5, 6)  # DoubleRow K-pair count per branch (ceil(n*300/256))
PRB = (0, 4, 9)  # weight pair-slot base per branch
NPR = 15  # total weight pair slots
KTOT = 12  # 128-row K-subtiles of the im2col per sample

NDUMMY = 6  # bf16 warm-up matmuls (N=512) to cover DMA lead-in + HAM

TRACE = False
LAST_RESULT = None

_built = None


def _build_bass():
    import concourse.mybir as mybir
    import concourse.tile as tile
    from concourse import bacc
    from contextlib import ExitStack

    f32 = mybir.dt.float32
    f8 = mybir.dt.float8e4
    bf16 = mybir.dt.bfloat16
    DR = mybir.MatmulPerfMode.DoubleRow

    nc = bacc.Bacc("TRN2", target_bir_lowering=False)
    xt_d = nc.dram_tensor("xt", (BPC, 128, KTOT, SEQP), f8, kind="ExternalInput")
    wq_d = nc.dram_tensor("wq", (128, 2, NPR, 2, 128), f8, kind="ExternalInput")
    bp_d = nc.dram_tensor("bp", (128, 3, 2), f32, kind="ExternalInput")
    out_d = nc.dram_tensor("out_t", (BPC, 128, 3, 2), f32, kind="ExternalOutput")

    with tile.TileContext(nc) as tc, ExitStack() as ctx:
        xpool = ctx.enter_context(tc.tile_pool(name="x", bufs=1))
        spool = ctx.enter_context(tc.tile_pool(name="stage", bufs=1))
        pspool = ctx.enter_context(tc.tile_pool(name="ps", bufs=1, space="PSUM"))

        # Warm-up fodder: PE busy from the first possible slot while DMAs
        # land (gpsimd memset so the DVE isn't on the critical path).
        dmy = spool.tile([128, 512], bf16, tag="dmy")
        nc.gpsimd.memset(dmy[:], 0.0)
        psd = pspool.tile([128, 512], f32, tag="dmy", bufs=1)
        for _ in range(NDUMMY):
            nc.tensor.matmul(psd[:], lhsT=dmy[:, :128], rhs=dmy[:], start=True,
                             stop=True)

        xs = [
            xpool.tile([128, KTOT, SEQP], f8, tag=f"x{s}", name=f"x{s}")
            for s in range(BPC)
        ]
        wt = [
            xpool.tile([128, NPR, 2, 128], f8, tag=f"w{dh}", name=f"w{dh}")
            for dh in range(2)
        ]
        bt = spool.tile([128, 3, 2], f32, tag="bt")
        nc.gpsimd.dma_start(bt[:], bp_d[:])

        def ldx(eng, s, a, b):
            eng.dma_start(xs[s][:, a:b], xt_d[s, :, a:b])

        def ldw(eng, dh, a, b):
            eng.dma_start(wt[dh][:, a:b], wq_d[:, dh, a:b])

        ldx(nc.sync, 0, 0, 4)
        ldw(nc.scalar, 0, 0, 4)
        ldx(nc.sync, 0, 4, 8)
        ldw(nc.scalar, 0, 4, 9)
        ldx(nc.sync, 0, 8, 12)
        ldw(nc.scalar, 0, 9, 15)
        ldw(nc.sync, 1, 9, 15)
        ldw(nc.scalar, 1, 0, 4)
        ldw(nc.scalar, 1, 4, 9)
        for s in range(1, BPC):
            ldx(nc.sync, s, 0, 6)
            ldx(nc.scalar, s, 6, 12)

        stage = spool.tile([128, BPC, 3, 2], f32, tag="stage")
        stage2 = spool.tile([128, BPC, 3, 2], f32, tag="stage2")

        def do_group(s, dh, br):
            n = NS[br]
            nw = SEQ - n  # windows the reference maxes over
            nmm = nw + (nw & 1)  # keep the moving count even
            np_ = NPAIRS[br]
            ps = pspool.tile([128, 512], f32, tag="ps", bufs=7,
                             name=f"ps_{s}_{dh}_{br}")
            for j in range(np_):
                nc.tensor.matmul(
                    ps[:, :nmm],
                    lhsT=wt[dh][:, PRB[br] + j, :, :],
                    rhs=xs[s][:, 2 * j : 2 * j + 2, :nmm],
                    start=(j == 0),
                    stop=(j == np_ - 1),
                    perf_mode=DR,
                )
            nc.vector.reduce_max(
                stage[:, s, br, dh : dh + 1],
                ps[:, :nw],
                axis=mybir.AxisListType.X,
            )

        for s in range(BPC):
            for dh in range(2):
                for br in range(3):
                    do_group(s, dh, br)
            # Per-sample epilogue + its own small contiguous output DMA.
            nc.vector.tensor_tensor(
                stage2[:, s], stage[:, s], bt[:], mybir.AluOpType.add
            )
            nc.vector.tensor_scalar_max(stage2[:, s], stage2[:, s], 0.0)
            (nc.sync, nc.scalar)[s % 2].dma_start(out_d[s], stage2[:, s])

    nc.compile()
    return nc


def _pack_inputs(input, W1, W2, W3, b1, b2, b3):
    # Host-materialized im2col: Xrep[b, k, t] = x[b, t + k//300, k%300],
    # 12 K-subtiles of 128 rows, SEQ padded to 400 with zeros, laid out
    # [sample, partition, subtile, t] so a sample loads in few big DMAs.
    f8 = ml_dtypes.float8_e4m3
    xt = np.zeros((B, EMB, SEQP), np.float32)
    xt[:, :, :SEQ] = np.asarray(input, np.float32).transpose(0, 2, 1)
    xrep = np.zeros((B, KTOT * 128, SEQP), np.float32)
    for j in range(5):
        xrep[:, j * EMB : (j + 1) * EMB, : SEQP - j] = xt[:, :, j:]
    xt8 = np.ascontiguousarray(
        xrep.reshape(B, KTOT, 128, SEQP).transpose(0, 2, 1, 3)
    ).astype(f8)

    # Weights: [partition, depth-half, pair-slot, pair-member, depth-col],
    # branch sections at PRB, rows zero-padded past each branch's n*300.
    wq = np.zeros((128, 2, NPR, 2, 128), np.float32)
    for br, (n, W) in enumerate(zip(NS, (W1, W2, W3))):
        Wt = np.asarray(W, np.float32).T  # [n*300, 256]
        for u in range(2 * NPAIRS[br]):
            rows = Wt[128 * u : min(128 * (u + 1), n * EMB)]
            if rows.shape[0] == 0:
                continue
            for dh in range(2):
                wq[: rows.shape[0], dh, PRB[br] + u // 2, u % 2, :] = (
                    rows[:, dh * 128 : (dh + 1) * 128]
                )
    wq8 = wq.astype(f8)

    bp = np.empty((128, 3, 2), np.float32)
    for br, b in enumerate((b1, b2, b3)):
        b = np.asarray(b, np.float32).reshape(DEPTH)
        for dh in range(2):
            bp[:, br, dh] = b[dh * 128 : (dh + 1) * 128]
    return xt8, wq8, bp


def kernel(input, W1, W2, W3, b1, b2, b3):
    global _built, LAST_RESULT
    from concourse.bass_utils import run_bass_kernel_spmd

    xt8, wq8, bp = _pack_inputs(input, W1, W2, W3, b1, b2, b3)

    if _built is None:
        _built = _build_bass()
    nc = _built

    in_maps = [
        {"xt": xt8[c * BPC : (c + 1) * BPC], "wq": wq8, "bp": bp}
        for c in range(NCORES)
    ]
    res = run_bass_kernel_spmd(
        nc, in_maps, core_ids=list(range(NCORES)), trace=TRACE
    )
    LAST_RESULT = res

    out = np.empty((B, 3 * DEPTH), np.float32)
    for c in range(NCORES):
        arr = res.results[c]["out_t"]  # [BPC, 128, 3, 2]
        out[c * BPC : (c + 1) * BPC] = arr.transpose(0, 2, 3, 1).reshape(BPC, 768)
    return out


# revision 17
# speedup vs baseline: 1.0236x; 1.0236x over previous
"""TextCNN-style conv layer (kernel sizes 3/4/5, EMB=300 -> DEPTH=256, bias,
ReLU, max-pool over time) as a Bass/Tile kernel for 8 Trainium2 NeuronCores.

Strategy: data-parallel over batch (8 samples per core), weights replicated.

Conv as dense-K matmuls over the im2col matrix Xrep[k, i] = x[i + k//300,
k%300] in fp8 e4m3 with DoubleRow matmuls: each MM contracts a PAIR of
128-row K-subtiles (virtual K=256, two fp8 weights per PE cell), so each
branch needs 4/5/6 pair-MMs = 15 per (sample, depth-half), 240 per core --
half the bf16 count, and HW paces them at the same ~165ns (N/2.4GHz), so
the PE stream floor drops from 79us to 39.5us.  e4m3 on both operands
measures L2 rel err 1.25e-2 on the seed-0 data (gate 2e-2).

Schedule (v5, from traces of the bf16/v1/v2/v3/v4 runs):
- Per-ring DMA completions pipeline at ~size/175GB/s + ~0.7us issue after
  a ~2.5us first-fill, so the prefetch uses few, deadline-ordered chunks:
  sample-0 x in 3 on sync, weights in 3+2 on scalar, sample-1 split
  across both rings, the rest alternating whole-sample loads.  (SWDGE
  for weights regressed 4us; bias only there.)
- 5 throwaway bf16 matmuls (N=512) cover the DMA lead-in from the first
  post-barrier slot (~7.5us) to the first real MM (~10.4us); small N=256
  fillers sit at the known supply seams inside sample 0/1 so the PE
  never idles long enough to hold the HAM clock-gate at K=4/8 (a ~1us
  gap there previously delayed K=8/8 by ~2.5us of half-clock matmuls).
- PSUM: 7 banks round-robin the 48 accumulation groups, 1 for dummies.
- Per-sample epilogue: bias-add + relu + own small contiguous output
  DMA, so the post-stream tail is one reduce + tiny DVE ops + one DMA.
- Fixed framework cost ~9us (semaphore-sweep teardown + entry barriers)
  is the same for any kernel here (57-59 sems/engine swept).
"""

import numpy as np
import ml_dtypes

B, SEQ, EMB = 64, 394, 300
DEPTH = 256
NCORES = 8
BPC = B // NCORES  # samples per core
SEQP = 400  # im2col free-dim padded (zeros) so all windows exist
NS = (3, 4, 5)
NPAIRS = (4, 5, 6)  # DoubleRow K-pair count per branch (ceil(n*300/256))
PRB = (0, 4, 9)  # weight pair-slot base per branch
NPR = 15  # total weight pair slots
KTOT = 12  # 128-row K-subtiles of the im2col per sample

NDUMMY = 9  # bf16 warm-up matmuls (N=512): runway to ~11.2us locks HAM
# at K=8/8 before the first supply stall (one gets dead-code-eliminated)

TRACE = False
LAST_RESULT = None

_built = None


def _build_bass():
    import concourse.mybir as mybir
    import concourse.tile as tile
    from concourse import bacc
    from contextlib import ExitStack

    f32 = mybir.dt.float32
    f8 = mybir.dt.float8e4
    bf16 = mybir.dt.bfloat16
    DR = mybir.MatmulPerfMode.DoubleRow

    nc = bacc.Bacc("TRN2", target_bir_lowering=False)
    xt_d = nc.dram_tensor("xt", (BPC, 128, KTOT, SEQP), f8, kind="ExternalInput")
    wq_d = nc.dram_tensor("wq", (128, 2, NPR, 2, 128), f8, kind="ExternalInput")
    bp_d = nc.dram_tensor("bp", (128, 3, 2), f32, kind="ExternalInput")
    out_d = nc.dram_tensor("out_t", (BPC, 128, 3, 2), f32, kind="ExternalOutput")

    with tile.TileContext(nc) as tc, ExitStack() as ctx:
        xpool = ctx.enter_context(tc.tile_pool(name="x", bufs=1))
        spool = ctx.enter_context(tc.tile_pool(name="stage", bufs=1))
        pspool = ctx.enter_context(tc.tile_pool(name="ps", bufs=1, space="PSUM"))

        # Warm-up fodder: PE busy from the first possible slot while DMAs
        # land (gpsimd memset so the DVE isn't on the critical path).
        dmy = spool.tile([128, 512], bf16, tag="dmy")
        nc.gpsimd.memset(dmy[:], 0.0)
        psd = pspool.tile([128, 512], f32, tag="dmy", bufs=1)
        for _ in range(NDUMMY):
            nc.tensor.matmul(psd[:], lhsT=dmy[:, :128], rhs=dmy[:], start=True,
                             stop=True)

        xs = [
            xpool.tile([128, KTOT, SEQP], f8, tag=f"x{s}", name=f"x{s}")
            for s in range(BPC)
        ]
        wt = [
            xpool.tile([128, NPR, 2, 128], f8, tag=f"w{dh}", name=f"w{dh}")
            for dh in range(2)
        ]
        bt = spool.tile([128, 3, 2], f32, tag="bt")
        nc.gpsimd.dma_start(bt[:], bp_d[:])

        def ldx(eng, s, a, b):
            eng.dma_start(xs[s][:, a:b], xt_d[s, :, a:b])

        def ldw(eng, dh, a, b):
            eng.dma_start(wt[dh][:, a:b], wq_d[:, dh, a:b])

        ldx(nc.sync, 0, 0, 6)
        ldw(nc.scalar, 0, 0, 4)
        ldx(nc.sync, 0, 6, 12)
        ldw(nc.scalar, 0, 4, 15)
        ldw(nc.sync, 1, 0, 4)
        ldw(nc.scalar, 1, 4, 15)
        for s in range(1, BPC):
            ldx(nc.sync, s, 0, 6)
            ldx(nc.scalar, s, 6, 12)

        stage = spool.tile([128, BPC, 3, 2], f32, tag="stage")
        stage2 = spool.tile([128, BPC, 3, 2], f32, tag="stage2")

        def do_group(s, dh, br):
            n = NS[br]
            nw = SEQ - n  # windows the reference maxes over
            nmm = nw + (nw & 1)  # keep the moving count even
            np_ = NPAIRS[br]
            ps = pspool.tile([128, 512], f32, tag="ps", bufs=7,
                             name=f"ps_{s}_{dh}_{br}")
            for j in range(np_):
                nc.tensor.matmul(
                    ps[:, :nmm],
                    lhsT=wt[dh][:, PRB[br] + j, :, :],
                    rhs=xs[s][:, 2 * j : 2 * j + 2, :nmm],
                    start=(j == 0),
                    stop=(j == np_ - 1),
                    perf_mode=DR,
                )
            nc.vector.reduce_max(
                stage[:, s, br, dh : dh + 1],
                ps[:, :nw],
                axis=mybir.AxisListType.X,
            )

        for s in range(BPC):
            for dh in range(2):
                for br in range(3):
                    do_group(s, dh, br)
            # Per-sample epilogue + its own small contiguous output DMA.
            nc.vector.tensor_tensor(
                stage2[:, s], stage[:, s], bt[:], mybir.AluOpType.add
            )
            nc.vector.tensor_scalar_max(stage2[:, s], stage2[:, s], 0.0)
            (nc.sync, nc.scalar)[s % 2].dma_start(out_d[s], stage2[:, s])

    nc.compile()
    return nc


def _pack_inputs(input, W1, W2, W3, b1, b2, b3):
    # Host-materialized im2col: Xrep[b, k, t] = x[b, t + k//300, k%300],
    # 12 K-subtiles of 128 rows, SEQ padded to 400 with zeros, laid out
    # [sample, partition, subtile, t] so a sample loads in few big DMAs.
    f8 = ml_dtypes.float8_e4m3
    xt = np.zeros((B, EMB, SEQP), np.float32)
    xt[:, :, :SEQ] = np.asarray(input, np.float32).transpose(0, 2, 1)
    xrep = np.zeros((B, KTOT * 128, SEQP), np.float32)
    for j in range(5):
        xrep[:, j * EMB : (j + 1) * EMB, : SEQP - j] = xt[:, :, j:]
    xt8 = np.ascontiguousarray(
        xrep.reshape(B, KTOT, 128, SEQP).transpose(0, 2, 1, 3)
    ).astype(f8)

    # Weights: [partition, depth-half, pair-slot, pair-member, depth-col],
    # branch sections at PRB, rows zero-padded past each branch's n*300.
    wq = np.zeros((128, 2, NPR, 2, 128), np.float32)
    for br, (n, W) in enumerate(zip(NS, (W1, W2, W3))):
        Wt = np.asarray(W, np.float32).T  # [n*300, 256]
        for u in range(2 * NPAIRS[br]):
            rows = Wt[128 * u : min(128 * (u + 1), n * EMB)]
            if rows.shape[0] == 0:
                continue
            for dh in range(2):
                wq[: rows.shape[0], dh, PRB[br] + u // 2, u % 2, :] = (
                    rows[:, dh * 128 : (dh + 1) * 128]
                )
    wq8 = wq.astype(f8)

    bp = np.empty((128, 3, 2), np.float32)
    for br, b in enumerate((b1, b2, b3)):
        b = np.asarray(b, np.float32).reshape(DEPTH)
        for dh in range(2):
            bp[:, br, dh] = b[dh * 128 : (dh + 1) * 128]
    return xt8, wq8, bp


def kernel(input, W1, W2, W3, b1, b2, b3):
    global _built, LAST_RESULT
    from concourse.bass_utils import run_bass_kernel_spmd

    xt8, wq8, bp = _pack_inputs(input, W1, W2, W3, b1, b2, b3)

    if _built is None:
        _built = _build_bass()
    nc = _built

    in_maps = [
        {"xt": xt8[c * BPC : (c + 1) * BPC], "wq": wq8, "bp": bp}
        for c in range(NCORES)
    ]
    res = run_bass_kernel_spmd(
        nc, in_maps, core_ids=list(range(NCORES)), trace=TRACE
    )
    LAST_RESULT = res

    out = np.empty((B, 3 * DEPTH), np.float32)
    for c in range(NCORES):
        arr = res.results[c]["out_t"]  # [BPC, 128, 3, 2]
        out[c * BPC : (c + 1) * BPC] = arr.transpose(0, 2, 3, 1).reshape(BPC, 768)
    return out


# revision 20
# speedup vs baseline: 1.0408x; 1.0167x over previous
"""TextCNN-style conv layer (kernel sizes 3/4/5, EMB=300 -> DEPTH=256, bias,
ReLU, max-pool over time) as a Bass/Tile kernel for 8 Trainium2 NeuronCores.

Strategy: data-parallel over batch (8 samples per core), weights replicated.

Conv as dense-K matmuls over the im2col matrix Xrep[k, i] = x[i + k//300,
k%300] in fp8 e4m3 with DoubleRow matmuls: each MM contracts a PAIR of
128-row K-subtiles (virtual K=256, two fp8 weights per PE cell), so each
branch needs 4/5/6 pair-MMs = 15 per (sample, depth-half), 240 per core --
half the bf16 count, and HW paces them at the same ~165ns (N/2.4GHz), so
the PE stream floor drops from 79us to 39.5us.  e4m3 on both operands
measures L2 rel err 1.25e-2 on the seed-0 data (gate 2e-2).

Schedule (v5, from traces of the bf16/v1/v2/v3/v4 runs):
- Per-ring DMA completions pipeline at ~size/175GB/s + ~0.7us issue after
  a ~2.5us first-fill, so the prefetch uses few, deadline-ordered chunks:
  sample-0 x in 3 on sync, weights in 3+2 on scalar, sample-1 split
  across both rings, the rest alternating whole-sample loads.  (SWDGE
  for weights regressed 4us; bias only there.)
- 5 throwaway bf16 matmuls (N=512) cover the DMA lead-in from the first
  post-barrier slot (~7.5us) to the first real MM (~10.4us); small N=256
  fillers sit at the known supply seams inside sample 0/1 so the PE
  never idles long enough to hold the HAM clock-gate at K=4/8 (a ~1us
  gap there previously delayed K=8/8 by ~2.5us of half-clock matmuls).
- PSUM: 7 banks round-robin the 48 accumulation groups, 1 for dummies.
- Per-sample epilogue: bias-add + relu + own small contiguous output
  DMA, so the post-stream tail is one reduce + tiny DVE ops + one DMA.
- Fixed framework cost ~9us (semaphore-sweep teardown + entry barriers)
  is the same for any kernel here (57-59 sems/engine swept).
"""

import numpy as np
import ml_dtypes

B, SEQ, EMB = 64, 394, 300
DEPTH = 256
NCORES = 8
BPC = B // NCORES  # samples per core
SEQP = 400  # im2col free-dim padded (zeros) so all windows exist
NS = (3, 4, 5)
NPAIRS = (4, 5, 6)  # DoubleRow K-pair count per branch (ceil(n*300/256))
PRB = (0, 4, 9)  # weight pair-slot base per branch
NPR = 15  # total weight pair slots
KTOT = 12  # 128-row K-subtiles of the im2col per sample

NDUMMY = 8  # bf16 warm-up matmuls (N=512): runway until the first weight
# chunk lands (~11us; one MM gets dead-code-eliminated)

TRACE = False
LAST_RESULT = None

_built = None


def _build_bass():
    import concourse.mybir as mybir
    import concourse.tile as tile
    from concourse import bacc
    from contextlib import ExitStack

    f32 = mybir.dt.float32
    f8 = mybir.dt.float8e4
    bf16 = mybir.dt.bfloat16
    DR = mybir.MatmulPerfMode.DoubleRow

    nc = bacc.Bacc("TRN2", target_bir_lowering=False)
    xt_d = nc.dram_tensor("xt", (BPC, 128, KTOT, SEQP), f8, kind="ExternalInput")
    wq_d = nc.dram_tensor("wq", (128, 2, NPR, 2, 128), f8, kind="ExternalInput")
    bp_d = nc.dram_tensor("bp", (128, 3, 2), f32, kind="ExternalInput")
    out_d = nc.dram_tensor("out_t", (BPC, 128, 3, 2), f32, kind="ExternalOutput")

    with tile.TileContext(nc) as tc, ExitStack() as ctx:
        xpool = ctx.enter_context(tc.tile_pool(name="x", bufs=1))
        spool = ctx.enter_context(tc.tile_pool(name="stage", bufs=1))
        pspool = ctx.enter_context(tc.tile_pool(name="ps", bufs=1, space="PSUM"))

        # Warm-up fodder: PE busy from the first possible slot while DMAs
        # land (gpsimd memset so the DVE isn't on the critical path).
        dmy = spool.tile([128, 512], bf16, tag="dmy")
        nc.gpsimd.memset(dmy[:], 0.0)
        psd = pspool.tile([128, 512], f32, tag="dmy", bufs=1)
        for _ in range(NDUMMY):
            nc.tensor.matmul(psd[:], lhsT=dmy[:, :128], rhs=dmy[:], start=True,
                             stop=True)

        xs = [
            xpool.tile([128, KTOT, SEQP], f8, tag=f"x{s}", name=f"x{s}")
            for s in range(BPC)
        ]
        wt = [
            xpool.tile([128, NPR, 2, 128], f8, tag=f"w{dh}", name=f"w{dh}")
            for dh in range(2)
        ]
        bt = spool.tile([128, 3, 2], f32, tag="bt")
        nc.gpsimd.dma_start(bt[:], bp_d[:])

        def ldx(eng, s, a, b):
            eng.dma_start(xs[s][:, a:b], xt_d[s, :, a:b])

        def ldw(eng, dh, a, b):
            eng.dma_start(wt[dh][:, a:b], wq_d[:, dh, a:b])

        ldx(nc.sync, 0, 0, 6)
        ldw(nc.scalar, 0, 0, 4)
        ldx(nc.sync, 0, 6, 12)
        ldw(nc.scalar, 0, 4, 15)
        ldw(nc.sync, 1, 0, 4)
        ldw(nc.scalar, 1, 4, 15)
        for s in range(1, BPC):
            ldx(nc.sync, s, 0, 6)
            ldx(nc.scalar, s, 6, 12)

        stage = spool.tile([128, BPC, 3, 2], f32, tag="stage")
        stage2 = spool.tile([128, BPC, 3, 2], f32, tag="stage2")

        def do_group(s, dh, br):
            n = NS[br]
            nw = SEQ - n  # windows the reference maxes over
            nmm = nw + (nw & 1)  # keep the moving count even
            np_ = NPAIRS[br]
            ps = pspool.tile([128, 512], f32, tag="ps", bufs=7,
                             name=f"ps_{s}_{dh}_{br}")
            for j in range(np_):
                nc.tensor.matmul(
                    ps[:, :nmm],
                    lhsT=wt[dh][:, PRB[br] + j, :, :],
                    rhs=xs[s][:, 2 * j : 2 * j + 2, :nmm],
                    start=(j == 0),
                    stop=(j == np_ - 1),
                    perf_mode=DR,
                )
            nc.vector.reduce_max(
                stage[:, s, br, dh : dh + 1],
                ps[:, :nw],
                axis=mybir.AxisListType.X,
            )
            # Fused bias-add + relu on the (otherwise idle) ACT engine.
            nc.scalar.activation(
                stage2[:, s, br, dh : dh + 1],
                stage[:, s, br, dh : dh + 1],
                mybir.ActivationFunctionType.Relu,
                bias=bt[:, br, dh : dh + 1],
            )

        for s in range(BPC):
            for dh in range(2):
                for br in range(3):
                    do_group(s, dh, br)
            # Per-sample output DMA once its 6 groups are staged.
            (nc.sync, nc.scalar)[s % 2].dma_start(out_d[s], stage2[:, s])

    nc.compile()
    return nc


def _pack_inputs(input, W1, W2, W3, b1, b2, b3):
    # Host-materialized im2col: Xrep[b, k, t] = x[b, t + k//300, k%300],
    # 12 K-subtiles of 128 rows, SEQ padded to 400 with zeros, laid out
    # [sample, partition, subtile, t] so a sample loads in few big DMAs.
    f8 = ml_dtypes.float8_e4m3
    xt = np.zeros((B, EMB, SEQP), np.float32)
    xt[:, :, :SEQ] = np.asarray(input, np.float32).transpose(0, 2, 1)
    xrep = np.zeros((B, KTOT * 128, SEQP), np.float32)
    for j in range(5):
        xrep[:, j * EMB : (j + 1) * EMB, : SEQP - j] = xt[:, :, j:]
    xt8 = np.ascontiguousarray(
        xrep.reshape(B, KTOT, 128, SEQP).transpose(0, 2, 1, 3)
    ).astype(f8)

    # Weights: [partition, depth-half, pair-slot, pair-member, depth-col],
    # branch sections at PRB, rows zero-padded past each branch's n*300.
    wq = np.zeros((128, 2, NPR, 2, 128), np.float32)
    for br, (n, W) in enumerate(zip(NS, (W1, W2, W3))):
        Wt = np.asarray(W, np.float32).T  # [n*300, 256]
        for u in range(2 * NPAIRS[br]):
            rows = Wt[128 * u : min(128 * (u + 1), n * EMB)]
            if rows.shape[0] == 0:
                continue
            for dh in range(2):
                wq[: rows.shape[0], dh, PRB[br] + u // 2, u % 2, :] = (
                    rows[:, dh * 128 : (dh + 1) * 128]
                )
    wq8 = wq.astype(f8)

    bp = np.empty((128, 3, 2), np.float32)
    for br, b in enumerate((b1, b2, b3)):
        b = np.asarray(b, np.float32).reshape(DEPTH)
        for dh in range(2):
            bp[:, br, dh] = b[dh * 128 : (dh + 1) * 128]
    return xt8, wq8, bp


def kernel(input, W1, W2, W3, b1, b2, b3):
    global _built, LAST_RESULT
    from concourse.bass_utils import run_bass_kernel_spmd

    xt8, wq8, bp = _pack_inputs(input, W1, W2, W3, b1, b2, b3)

    if _built is None:
        _built = _build_bass()
    nc = _built

    in_maps = [
        {"xt": xt8[c * BPC : (c + 1) * BPC], "wq": wq8, "bp": bp}
        for c in range(NCORES)
    ]
    res = run_bass_kernel_spmd(
        nc, in_maps, core_ids=list(range(NCORES)), trace=TRACE
    )
    LAST_RESULT = res

    out = np.empty((B, 3 * DEPTH), np.float32)
    for c in range(NCORES):
        arr = res.results[c]["out_t"]  # [BPC, 128, 3, 2]
        out[c * BPC : (c + 1) * BPC] = arr.transpose(0, 2, 3, 1).reshape(BPC, 768)
    return out
